# revision 50
# baseline (speedup 1.0000x reference)
"""Self-contained Trainium2 Bass kernel for a 6-layer dense transformer.

Model (from reference): DIM=1024, DEPTH=6, HEADS=16, FF=4096, x [2,1024,1024],
relative_position_bias [1,16,1024,1024], pre-norm attention+FFN, exact GELU.

Strategy: sequence-parallel over 8 NeuronCores. Rows = flatten(batch, seq) =
2048; each core owns 256 rows (batch b = core//4, seq chunk core%4).
Activations are CHANNEL-major (transposed: [D, rows]) so every matmul
contraction lands on the partition axis with zero on-chip transposes.

v2 over the original baseline:
  - Weights host-precast/prepacked: w_qkv / w_out in fp8e4 driven through
    DoubleRow matmuls (2 k-tiles per pass, 0.5 cyc/row); w1 / w2 in bf16.
  - exp(bias) precomputed on host (bf16), DMAed once into SBUF.
  - Per-layer K/V AllGather payload in fp8e4 (2 MiB out vs 4 MiB bf16).
  - Partition-broadcasts (LN mu/rstd, softmax 1/den) via stride-0 DMA
    replication instead of PSUM matmuls - frees PSUM banks and PE time.
  - Softmax exp in [128, 1024] batches; scores PSUM tiles span 2 banks.
  - Per-layer params packed into one [128, 80] f32 DMA.
"""
import sys
sys.path.insert(0, "/opt/trn_rl_repo")

import numpy as np
import ml_dtypes

import concourse.bass as bass
import concourse.tile as tile
from concourse import bacc, mybir

P = 128
D = 1024
DT = 8            # D / P tiles
DEPTH = 6
HEADS = 16
DH = 64
FF = 4096
FFT = 32          # FF / P tiles
R = 256           # rows per core
B = 2
SEQ = 1024
N_CORES = 8
EPS = 1e-5
SCALE = DH ** -0.5
RG = [[0, 1, 2, 3], [4, 5, 6, 7]]

F32 = mybir.dt.float32
BF16 = mybir.dt.bfloat16
FP8 = mybir.dt.float8e4
AX = mybir.AluOpType
AF = mybir.ActivationFunctionType
DR = mybir.MatmulPerfMode.DoubleRow

NP_BF16 = ml_dtypes.bfloat16
NP_FP8 = ml_dtypes.float8_e4m3

KV_K = D * R          # elems in k^T region of the gather payload


def _bcast_mid(ap, n):
    """View a [P, N] AP as [P, n, N] with a 0-stride middle dim."""
    return bass.AP(tensor=ap.tensor, offset=ap.offset,
                   ap=[list(ap.ap[0]), [0, n], list(ap.ap[1])])


def _prow(ap, p):
    """View a [1, N] AP as [p, N] with 0-stride partitions (DMA bcast src)."""
    return bass.AP(tensor=ap.tensor, offset=ap.offset,
                   ap=[[0, p], list(ap.ap[-1])])


def build_nc():
    nc = bacc.Bacc("TRN2", target_bir_lowering=False, debug=False,
                   num_devices=N_CORES)

    xT_ext = nc.dram_tensor("xT", [D, R], F32, kind="ExternalInput")
    eb_ext = nc.dram_tensor("eb", [P, HEADS, DT, R], BF16, kind="ExternalInput")
    w_qkv_ext = nc.dram_tensor("w_qkv", [DEPTH, 6, P, DT, 4 * P], FP8,
                               kind="ExternalInput")
    w_out_ext = nc.dram_tensor("w_out", [DEPTH, 2, P, DT, 4 * P], FP8,
                               kind="ExternalInput")
    w1_ext = nc.dram_tensor("w1", [DEPTH, 8, P, DT, 4 * P], BF16,
                            kind="ExternalInput")
    w2_ext = nc.dram_tensor("w2", [DEPTH, 4, 2, P, 16, 2 * P], BF16,
                            kind="ExternalInput")
    pk_ext = nc.dram_tensor("pk", [DEPTH, P, 80], F32, kind="ExternalInput")
    outT_ext = nc.dram_tensor("outT", [D, R], F32, kind="ExternalOutput")

    from contextlib import ExitStack
    with tile.TileContext(nc) as tc, ExitStack() as ctx:
        ep = ctx.enter_context
        singles = ep(tc.tile_pool(name="singles", bufs=1))
        params = ep(tc.tile_pool(name="params", bufs=2))
        statp = ep(tc.tile_pool(name="stat", bufs=2))
        hTp = ep(tc.tile_pool(name="hTp", bufs=1))
        h2p = ep(tc.tile_pool(name="h2p", bufs=1))
        qTp = ep(tc.tile_pool(name="qTp", bufs=1))
        ktp = ep(tc.tile_pool(name="ktp", bufs=1))
        vpp = ep(tc.tile_pool(name="vpp", bufs=1))
        attnp = ep(tc.tile_pool(name="attnp", bufs=2))
        oTp = ep(tc.tile_pool(name="oTp", bufs=1))
        gTp = ep(tc.tile_pool(name="gTp", bufs=1))
        wcp = ep(tc.tile_pool(name="wcp", bufs=2))
        vecp = ep(tc.tile_pool(name="vecp", bufs=4))
        vecp16 = ep(tc.tile_pool(name="vecp16", bufs=2))
        psmm = ep(tc.tile_pool(name="psmm", bufs=2, space="PSUM"))
        pss = ep(tc.tile_pool(name="pss", bufs=2, space="PSUM"))
        psav = ep(tc.tile_pool(name="psav", bufs=1, space="PSUM"))
        psst = ep(tc.tile_pool(name="psst", bufs=1, space="PSUM"))
        dram = ep(tc.tile_pool(name="dram", bufs=2, space="DRAM"))
        if True:
            # ---- persistent tiles ----
            xT = singles.tile([P, DT, R], F32, tag="xT")
            EB = singles.tile([P, HEADS, DT, R], BF16, tag="EB")
            ones_red = singles.tile([P, 1], BF16, tag="ones_red")
            ones_k1 = singles.tile([1, P], BF16, tag="ones_k1")
            nc.vector.memset(ones_red[:], 1.0)
            nc.vector.memset(ones_k1[:], 1.0)

            nc.sync.dma_start(
                out=xT[:], in_=xT_ext.ap().rearrange("(t p) r -> p t r", p=P))

            # EB = exp(bias^T) resident bf16, host-precomputed. Loaded inside
            # layer 0 to overlap the first AllGather.
            eb_emitted = [False]

            def emit_eb_load():
                if eb_emitted[0]:
                    return
                eb_emitted[0] = True
                for g in range(4):
                    nc.sync.dma_start(out=EB[:, g * 4:(g + 1) * 4],
                                      in_=eb_ext.ap()[:, g * 4:(g + 1) * 4])

            def ln_alloc(tag):
                xb = statp.tile([P, DT, R], BF16, tag="stat", name=f"xb_{tag}")
                ps_st = psst.tile([33, R], F32, tag="st", name=f"st_{tag}")
                return xb, ps_st, tag

            def ln_contrib(st, t):
                """Accumulate LN stats for channel-tile t of xT."""
                xb, ps_st, tag = st
                sq = statp.tile([P, R], BF16, tag="stat2", name=f"sq_{tag}_{t}")
                nc.vector.tensor_copy(xb[:, t], xT[:, t])
                nc.vector.tensor_mul(sq[:], xb[:, t], xb[:, t])
                nc.tensor.matmul(ps_st[0:1], ones_red[:], xb[:, t],
                                 start=(t == 0), stop=(t == DT - 1))
                nc.tensor.matmul(ps_st[32:33], ones_red[:], sq[:],
                                 start=(t == 0), stop=(t == DT - 1))

            def ln_finish(st, pk, g_off, b_off, out_hT, tag):
                """LN over channel (partition) axis of xT -> out_hT.

                out = ((x - bcast(mu)) * g) * bcast(rstd) + b
                """
                xb, ps_st, _ = st
                mu = vecp.tile([1, R], F32, tag="vec", name=f"mu_{tag}")
                var = vecp.tile([1, R], F32, tag="vec", name=f"var_{tag}")
                ms = vecp.tile([1, R], F32, tag="vec", name=f"ms_{tag}")
                rstd = vecp.tile([1, R], F32, tag="vec", name=f"rstd_{tag}")
                mr16 = vecp16.tile([1, 2, R], BF16, tag="vec16", name=f"mr16_{tag}")
                nc.vector.tensor_scalar_mul(mu[:], ps_st[0:1], 1.0 / D)
                nc.vector.tensor_scalar_mul(var[:], ps_st[32:33], 1.0 / D)
                nc.vector.tensor_mul(ms[:], mu[:], mu[:])
                nc.vector.tensor_sub(var[:], var[:], ms[:])
                nc.vector.tensor_scalar_add(var[:], var[:], EPS)
                nc.scalar.activation(var[:], var[:], AF.Sqrt)
                nc.vector.reciprocal(rstd[:], var[:])
                nc.vector.tensor_copy(mr16[:, 0], mu[:])
                nc.vector.tensor_copy(mr16[:, 1], rstd[:])
                ps_bc = pss.tile([P, 4, R], F32, tag="ss", name=f"psbc_{tag}")
                nc.tensor.matmul(ps_bc[:, 0:2].rearrange("p a b -> p (a b)"),
                                 ones_k1[:],
                                 mr16.rearrange("1 a b -> 1 (a b)"),
                                 start=True, stop=True)
                mrb = statp.tile([P, 2, R], BF16, tag="statv", name=f"mrb_{tag}")
                nc.vector.tensor_copy(mrb[:], ps_bc[:, 0:2])
                nc.vector.tensor_sub(xb[:], xb[:], _bcast_mid(mrb[:, 0], DT))
                for t in range(DT):
                    nc.vector.scalar_tensor_tensor(
                        out=xb[:, t], in0=xb[:, t],
                        scalar=pk[:, g_off + t:g_off + t + 1],
                        in1=mrb[:, 1], op0=AX.mult, op1=AX.mult)
                for t in range(DT):
                    nc.vector.tensor_scalar_add(out_hT[:, t], xb[:, t],
                                                pk[:, b_off + t:b_off + t + 1])

            ln1_st = None
            for l in range(DEPTH):
                pk = params.tile([P, 80], F32, tag="pk", name=f"pk_{l}")
                nc.sync.dma_start(out=pk[:], in_=pk_ext.ap()[l])

                # ---- LN1 (stats carried from prev mm2 epilogue) ----
                if l == 0:
                    ln1_st = ln_alloc("l0a")
                    for t in range(DT):
                        ln_contrib(ln1_st, t)
                h8 = hTp.tile([P, DT, R], FP8, tag="hT", name=f"h8_{l}")
                ln_finish(ln1_st, pk, 0, 8, h8, f"l{l}a")

                qT = qTp.tile([P, DT, R], FP8, tag="qT", name=f"qT_{l}")

                # ---- gather h (fp8, 1 MiB out) instead of K/V (2 MiB) ----
                h_in = dram.tile([KV_K], FP8, tag="h_in", name=f"hi_{l}")
                h_out = dram.tile([4, KV_K], FP8, tag="h_out", name=f"ho_{l}")
                nc.sync.dma_start(
                    out=h_in[:].rearrange("(p n) -> p n", p=P), in_=h8[:])
                emit_eb_load()
                nc.gpsimd.collective_compute(
                    "AllGather", AX.bypass, replica_groups=RG,
                    ins=[h_in[:]], outs=[h_out[:]])

                def proj8(wc, rhs, c0, tag, dst=None):
                    """Two 128-col output tiles from one [P, 2, R] dual-group
                    psum round. Copies into dst cols [c0, c0+2) if given, else
                    returns the psum tile."""
                    ps2 = psmm.tile([P, 2, R], F32, tag="mm", name=f"pp_{tag}")
                    for sub in range(2):
                        off = (c0 % 4) * P + sub * P
                        for t in range(DT // 2):
                            nc.tensor.matmul(
                                ps2[:, sub], wc[:, 2 * t:2 * t + 2, off:off + P],
                                rhs[:, 2 * t:2 * t + 2],
                                start=(t == 0), stop=(t == DT // 2 - 1),
                                perf_mode=DR)
                    if dst is not None:
                        nc.vector.tensor_copy(dst[:, c0:c0 + 2], ps2[:])
                    return ps2

                for ch in (0, 1):           # q cols 0..1023 (overlaps AG)
                    wc = wcp.tile([P, DT, 4 * P], FP8, tag="wc8",
                                  name=f"wcq_{l}_{ch}")
                    nc.sync.dma_start(out=wc[:], in_=w_qkv_ext.ap()[l, ch])
                    for g in range(2):
                        proj8(wc, h8, ch * 4 + 2 * g, f"q_{l}_{ch}_{g}", dst=qT)

                # gathered h -> SBUF (Act HWDGE queue; SP keeps streaming
                # weights), then full-seq K/V via fp8 DR matmuls locally.
                hF = ktp.tile([P, DT, SEQ], FP8, tag="hF", name=f"hF_{l}")
                for r in range(4):
                    nc.scalar.dma_start(
                        out=hF[:, :, r * R:(r + 1) * R],
                        in_=h_out[r].rearrange("(p t r) -> p t r", p=P, r=R))

                KT2 = ktp.tile([P, DT, SEQ], FP8, tag="KT2", name=f"KT2_{l}")
                Vp = vpp.tile([P, DT, HEADS, DH + 1], FP8, tag="Vp", name=f"Vp_{l}")
                nc.vector.memset(Vp[:, :, :, DH:DH + 1], 1.0)
                for ch in (2, 3):           # k cols -> KT2 (channel-major)
                    wc = wcp.tile([P, DT, 4 * P], FP8, tag="wc8",
                                  name=f"wck_{l}_{ch}")
                    nc.sync.dma_start(out=wc[:], in_=w_qkv_ext.ap()[l, ch])
                    for sub in range(4):
                        c = (ch - 2) * 4 + sub
                        psk = pss.tile([P, 4, R], F32, tag="ss",
                                       name=f"psk_{l}_{ch}_{sub}")
                        for half in range(2):
                            for t in range(DT // 2):
                                nc.tensor.matmul(
                                    psk[:, 2 * half:2 * half + 2].rearrange(
                                        "p a b -> p (a b)"),
                                    wc[:, 2 * t:2 * t + 2, sub * P:(sub + 1) * P],
                                    hF[:, 2 * t:2 * t + 2,
                                       half * 2 * R:(half + 1) * 2 * R],
                                    start=(t == 0), stop=(t == DT // 2 - 1),
                                    perf_mode=DR)
                        nc.vector.tensor_copy(
                            KT2[:, c], psk.rearrange("p a b -> p (a b)"))

                for ch in (4, 5):           # v cols -> Vp (row-major)
                    wc = wcp.tile([P, DT, 4 * P], FP8, tag="wc8",
                                  name=f"wcv_{l}_{ch}")
                    nc.sync.dma_start(out=wc[:], in_=w_qkv_ext.ap()[l, ch])
                    for kb in range(DT):
                        ps2 = psmm.tile([P, 2, R], F32, tag="mm",
                                        name=f"psv_{l}_{ch}_{kb}")
                        for t in range(DT // 2):
                            nc.tensor.matmul(
                                ps2.rearrange("p a b -> p (a b)"),
                                hF[:, 2 * t:2 * t + 2, kb * P:(kb + 1) * P],
                                wc[:, 2 * t:2 * t + 2],
                                start=(t == 0), stop=(t == DT // 2 - 1),
                                perf_mode=DR)
                        hh = (ch - 4) * 8
                        nc.scalar.activation(
                            Vp[:, kb, hh:hh + 8, 0:DH],
                            ps2.rearrange("p a b -> p (a b)").rearrange(
                                "p (h j) -> p h j", j=DH), AF.Copy)

                # ---- attention per head ----
                oT = oTp.tile([P, DT, R], FP8, tag="oT", name=f"oT_{l}")
                pso2 = psav.tile([P, 2, R], F32, tag="av", name=f"pso2_{l}")
                for h in range(HEADS):
                    pb = (h % 2) * DH
                    ht = h // 2
                    at = attnp.tile([P, DT, R], BF16, tag="attn", name=f"at_{l}_{h}")
                    ps_o = pso2[0:DH + 1, h % 2]
                    for half in range(2):
                        ps_s = pss.tile([P, 4, R], F32, tag="ss",
                                        name=f"pss_{l}_{h}_{half}")
                        for j in range(4):
                            kt = half * 4 + j
                            nc.tensor.matmul(
                                ps_s[:, j],
                                KT2[pb:pb + DH, ht, kt * P:(kt + 1) * P],
                                qT[pb:pb + DH, ht],
                                start=True, stop=True)
                        nc.scalar.activation(
                            at[:, half * 4:(half + 1) * 4].rearrange(
                                "p a b -> p (a b)"),
                            ps_s.rearrange("p a b -> p (a b)"),
                            AF.Exp, scale=SCALE)
                    nc.vector.tensor_mul(at[:], at[:], EB[:, h])
                    for kt in range(DT):
                        nc.tensor.matmul(ps_o, Vp[:, kt, h], at[:, kt],
                                         start=(kt == 0), stop=(kt == DT - 1))
                    rec16 = vecp16.tile([1, R], BF16, tag="vec16b", name=f"rec16_{l}_{h}")
                    with nc.allow_low_precision(reason="softmax denom bf16"):
                        nc.vector.reciprocal(rec16[:], pso2[DH:DH + 1, h % 2])
                    ps_b = psmm.tile([P, 2, R], F32, tag="mm", name=f"ps_b_{l}_{h}")
                    nc.tensor.matmul(ps_b[0:DH, 0], ones_k1[0:1, 0:DH], rec16[:],
                                     start=True, stop=True)
                    rb16 = vecp16.tile([DH, R], BF16, tag="vecrb", name=f"rb_{l}_{h}")
                    nc.vector.tensor_copy(rb16[:], ps_b[0:DH, 0])
                    nc.vector.tensor_mul(oT[pb:pb + DH, ht], ps_o[0:DH], rb16[:])

                # ---- attn out projection (fp8 DR) + residual (+LN2 stats) ----
                ln2_st = ln_alloc(f"l{l}b")
                for ch in range(2):
                    wc = wcp.tile([P, DT, 4 * P], FP8, tag="wc8",
                                  name=f"wco_{l}_{ch}")
                    nc.sync.dma_start(out=wc[:], in_=w_out_ext.ap()[l, ch])
                    for g in range(2):
                        c0 = ch * 4 + 2 * g
                        ps2 = proj8(wc, oT, c0, f"o_{l}_{ch}_{g}")
                        for sub in range(2):
                            c = c0 + sub
                            nc.vector.scalar_tensor_tensor(
                                out=xT[:, c], in0=ps2[:, sub],
                                scalar=pk[:, 32 + c:33 + c],
                                in1=xT[:, c], op0=AX.add, op1=AX.add)
                            ln_contrib(ln2_st, c)

                # ---- LN2 + FFN (bf16) ----
                h2 = h2p.tile([P, DT, R], BF16, tag="h2", name=f"h2_{l}")
                ln_finish(ln2_st, pk, 16, 24, h2, f"l{l}b")

                gT = gTp.tile([P, FFT, R], BF16, tag="gT", name=f"gT_{l}")
                for ch in range(8):
                    wc = wcp.tile([P, DT, 4 * P], BF16, tag="wcb",
                                  name=f"wc1_{l}_{ch}")
                    nc.sync.dma_start(out=wc[:], in_=w1_ext.ap()[l, ch])
                    for g in range(2):
                        ps2 = psmm.tile([P, 2, R], F32, tag="mm",
                                        name=f"psf_{l}_{ch}_{g}")
                        for sub in range(2):
                            off = 2 * g * P + sub * P
                            for kt in range(DT):
                                nc.tensor.matmul(
                                    ps2[:, sub], wc[:, kt, off:off + P],
                                    h2[:, kt], start=(kt == 0),
                                    stop=(kt == DT - 1))
                        for sub in range(2):
                            f = ch * 4 + 2 * g + sub
                            nc.scalar.activation(gT[:, f], ps2[:, sub], AF.Gelu,
                                                 bias=pk[:, 48 + f:49 + f])

                if l < DEPTH - 1:
                    ln1_st = ln_alloc(f"l{l + 1}a")
                for cp in range(4):
                    wc2s = []
                    for ktg in range(2):
                        wc = wcp.tile([P, 16, 2 * P], BF16, tag="wc2b",
                                      name=f"wc2_{l}_{cp}_{ktg}")
                        nc.sync.dma_start(out=wc[:], in_=w2_ext.ap()[l, cp, ktg])
                        wc2s.append(wc)
                    ps2 = psmm.tile([P, 2, R], F32, tag="mm",
                                    name=f"ps_mm2_{l}_{cp}")
                    for sub in range(2):
                        for ktg in range(2):
                            for k16 in range(16):
                                nc.tensor.matmul(
                                    ps2[:, sub],
                                    wc2s[ktg][:, k16, sub * P:(sub + 1) * P],
                                    gT[:, ktg * 16 + k16],
                                    start=(ktg == 0 and k16 == 0),
                                    stop=(ktg == 1 and k16 == 15))
                    for sub in range(2):
                        c = cp * 2 + sub
                        nc.vector.scalar_tensor_tensor(
                            out=xT[:, c], in0=ps2[:, sub],
                            scalar=pk[:, 40 + c:41 + c],
                            in1=xT[:, c], op0=AX.add, op1=AX.add)
                        if l < DEPTH - 1:
                            ln_contrib(ln1_st, c)

            nc.sync.dma_start(
                out=outT_ext.ap().rearrange("(t p) r -> p t r", p=P), in_=xT[:])

    nc.compile()
    return nc


def make_in_maps(inputs):
    x = np.ascontiguousarray(np.asarray(inputs["x"], dtype=np.float32))
    bias = np.asarray(inputs["relative_position_bias"], dtype=np.float32)

    def pack(w, nch, np_dt):
        # [DEPTH, 128*DT rows, 512*nch cols] -> [DEPTH, nch, 128, DT, 512]
        w = np.asarray(w, dtype=np.float32)
        return np.ascontiguousarray(
            w.reshape(DEPTH, DT, P, nch, 4 * P).transpose(0, 3, 2, 1, 4)
            .astype(np_dt))

    # w2: [DEPTH, FF, D] -> [DEPTH, cp(4 x 256cols), ktg(2), 128, 16kt, 256]
    w2 = np.asarray(inputs["w2"], dtype=np.float32)
    w2p = np.ascontiguousarray(
        w2.reshape(DEPTH, 2, 16, P, 4, 2 * P).transpose(0, 4, 1, 3, 2, 5)
        .astype(NP_BF16))

    def col(v):
        # [DEPTH, D] -> [DEPTH, P, DT] with channel d = t*128+p at [p, t]
        return np.asarray(v, dtype=np.float32).reshape(DEPTH, DT, P).transpose(0, 2, 1)

    pk = np.empty((DEPTH, P, 80), dtype=np.float32)
    pk[:, :, 0:8] = col(inputs["ln1_g"])
    pk[:, :, 8:16] = col(inputs["ln1_b"])
    pk[:, :, 16:24] = col(inputs["ln2_g"])
    pk[:, :, 24:32] = col(inputs["ln2_b"])
    pk[:, :, 32:40] = col(inputs["b_out"])
    pk[:, :, 40:48] = col(inputs["b2"])
    pk[:, :, 48:80] = np.asarray(inputs["b1"], dtype=np.float32).reshape(
        DEPTH, FFT, P).transpose(0, 2, 1)

    shared = {
        "w_qkv": pack(inputs["w_qkv"], 6, NP_FP8),
        "w_out": pack(inputs["w_out"], 2, NP_FP8),
        "w1": pack(inputs["w1"], 8, NP_BF16),
        "w2": w2p,
        "pk": np.ascontiguousarray(pk),
    }
    in_maps = []
    for c in range(N_CORES):
        b, s0 = c // 4, (c % 4) * R
        m = dict(shared)
        m["xT"] = np.ascontiguousarray(x[b, s0:s0 + R, :].T)
        # EB[p, h, t, r] = exp(bias[0, h, s0+r, t*128+p])
        bt = np.exp(bias[0, :, s0:s0 + R, :])            # [H, 256 q, 1024 keys]
        m["eb"] = np.ascontiguousarray(
            bt.reshape(HEADS, R, DT, P).transpose(3, 0, 2, 1).astype(NP_BF16))
        in_maps.append(m)
    return in_maps


_NC_CACHE = {}


def kernel(**inputs):
    from concourse.bass_utils import run_bass_kernel_spmd
    if "nc" not in _NC_CACHE:
        _NC_CACHE["nc"] = build_nc()
    nc = _NC_CACHE["nc"]
    in_maps = make_in_maps(inputs)
    res = run_bass_kernel_spmd(nc, in_maps, core_ids=list(range(N_CORES)))
    out = np.empty((B, SEQ, D), dtype=np.float32)
    for c in range(N_CORES):
        b, s0 = c // 4, (c % 4) * R
        out[b, s0:s0 + R, :] = res.results[c]["outT"].T
    return out
